# revision 10
# baseline (speedup 1.0000x reference)
"""Trainium2 Bass kernel for span-attention pooling.

Problem shapes (hardcoded):
  x: [B=2, T=512, E=1024] f32, W: [1024, 1] f32, b: [1] f32,
  start/end: [S=2048] i32.  Output: [B, S, E] f32.

Math: out[b,s,:] = sum_{t=start[s]}^{end[s]} q[b,t] * x[b,t,:] / sum q[b,t]
with q = exp(relu(x @ W + b)).  (Equivalent to the reference's per-span
softmax over head scores, since spans are contiguous token ranges and
clamped/invalid positions carry zero weight.)

Sharding: 8 cores = (batch b in {0,1}) x (span quarter of 512 spans).
Each core computes out[b, q*512:(q+1)*512, :].
"""

import numpy as np

import concourse.bass as bass
import concourse.tile as tile
from concourse import bacc, mybir
from concourse import bass_utils

B, T, E = 2, 512, 1024
S, A = 2048, 30
N_CORES = 8
SQ = S // 4  # spans per core
TCH = T // 128  # T chunks of 128 partitions
SCH = SQ // 128  # span chunks of 128 partitions

F32 = mybir.dt.float32
F32R = mybir.dt.float32r
I32 = mybir.dt.int32

# dtype used for the main matmul operands. float32r runs the PE at
# full rate (1 cyc/row at N>=512) with ~tf32 precision (~1.6e-4 rel,
# HW-measured); float32 is exact but 4x slower. The walrus verifier
# requires f32r matmul inputs to be *produced* as f32r, so the tiles
# feeding the matmul are typed MM_DT and bitcast to f32 for DVE reads.
MM_DT = F32R


def _f32view(ap):
    if MM_DT == F32:
        return ap
    return ap.bitcast(F32)


def _build_body(tc, out_d, x_d, w_d, b_d, st_d, en_d):
    nc = tc.nc
    AF = mybir.ActivationFunctionType
    OP = mybir.AluOpType

    with (
        tc.tile_pool(name="main", bufs=1) as mainp,
        tc.tile_pool(name="outp", bufs=2) as outp,
        tc.tile_pool(name="psum", bufs=2, space="PSUM") as psp,
        tc.tile_pool(name="scr", bufs=1) as scrp,
    ):
        # x chunks: [128, TCH, E].
        xt = mainp.tile([128, TCH, E], MM_DT)
        for i in range(TCH):
            nc.sync.dma_start(xt[:, i, :], x_d[128 * i : 128 * (i + 1), :])

        # ones tile for the Z (softmax denominator) matmul. 64 wide:
        # f32r matmuls reject tiny moving free dims (N=1 fails the ISA
        # check), and 64 costs nothing. memset can't write f32r, so
        # memset f32 then tensor_copy into the f32r tile.
        ones_f = mainp.tile([128, 64], F32)
        nc.vector.memset(ones_f[:], 1.0)
        ones_r = mainp.tile([128, 64], MM_DT)
        nc.vector.tensor_copy(ones_r[:], ones_f[:])

        # Small loads: W row, bias, start/end rows.
        w_row = mainp.tile([1, E], F32)
        nc.sync.dma_start(w_row[0:1, :], w_d.rearrange("(o e) -> o e", o=1))
        b_row = mainp.tile([1, 1], F32)
        nc.sync.dma_start(b_row[0:1, :], b_d.rearrange("(o e) -> o e", o=1))
        st_i = mainp.tile([1, SQ], I32)
        nc.sync.dma_start(st_i[0:1, :], st_d.rearrange("(o s) -> o s", o=1))
        en_i = mainp.tile([1, SQ], I32)
        nc.sync.dma_start(en_i[0:1, :], en_d.rearrange("(o s) -> o s", o=1))

        # int -> float casts (values < 2^24, exact).
        st_f = mainp.tile([1, SQ], F32)
        nc.vector.tensor_copy(st_f[0:1, :], st_i[0:1, :])
        en_f = mainp.tile([1, SQ], F32)
        nc.vector.tensor_copy(en_f[0:1, :], en_i[0:1, :])

        # Broadcast along partitions (GPSIMD).
        wb = mainp.tile([128, E], F32)
        nc.gpsimd.partition_broadcast(wb[:], w_row[0:1, :])
        bb = mainp.tile([128, 1], F32)
        nc.gpsimd.partition_broadcast(bb[:], b_row[0:1, :])
        stb = mainp.tile([128, SQ], F32)
        nc.gpsimd.partition_broadcast(stb[:], st_f[0:1, :])
        enb = mainp.tile([128, SQ], F32)
        nc.gpsimd.partition_broadcast(enb[:], en_f[0:1, :])

        # t-values per partition for each T chunk: t = 128*i + p.
        t_i = mainp.tile([128, TCH], I32)
        nc.gpsimd.iota(t_i[:], pattern=[[128, TCH]], base=0, channel_multiplier=1)
        t_f = mainp.tile([128, TCH], F32)
        nc.vector.tensor_copy(t_f[:], t_i[:])

        # Per T-chunk: head score h, q = exp(relu(h)), and
        # maskq[t, s] = (start[s] <= t <= end[s]) * q[t]  (transposed layout).
        q_col = mainp.tile([128, TCH], F32)
        maskq = mainp.tile([128, TCH, SQ], MM_DT)
        scr = scrp.tile([128, E], F32)
        m2 = scrp.tile([128, SQ], F32)
        h = scrp.tile([128, TCH], F32)
        for i in range(TCH):
            # h = sum_e x[t, e] * W[e]   (scalar_tensor_tensor with accum;
            # InstTensorTensorReduce faults on this HW path)
            nc.vector.scalar_tensor_tensor(
                scr[:],
                _f32view(xt[:, i, 0:E]),
                1.0,
                wb[:],
                op0=OP.mult,
                op1=OP.mult,
                accum_out=h[:, i : i + 1],
            )
            # q = exp(relu(h)) = max(exp(h + b), 1)   (bias folded into Exp)
            nc.scalar.activation(
                q_col[:, i : i + 1], h[:, i : i + 1], AF.Exp, bias=bb[:, 0:1]
            )
            nc.vector.tensor_scalar_max(q_col[:, i : i + 1], q_col[:, i : i + 1], 1.0)
            # m2 = (end >= t) * q
            nc.vector.tensor_scalar(
                m2[:],
                enb[:],
                t_f[:, i : i + 1],
                q_col[:, i : i + 1],
                op0=OP.is_ge,
                op1=OP.mult,
            )
            # maskq = (start <= t) * m2
            nc.vector.scalar_tensor_tensor(
                maskq[:, i, :],
                stb[:],
                t_f[:, i : i + 1],
                m2[:],
                op0=OP.is_le,
                op1=OP.mult,
            )

        # Main matmuls per span chunk j:
        #   out_psum[s, e] = sum_t maskq[t, s] * x[t, e]   (two 512-wide halves)
        #   Z[s]           = sum_t maskq[t, s] * 1
        for j in range(SCH):
            po0 = psp.tile([128, 512], F32, tag="po0")
            po1 = psp.tile([128, 512], F32, tag="po1")
            zp = psp.tile([128, 64], F32, tag="zp")
            for i in range(TCH):
                lhsT = maskq[:, i, 128 * j : 128 * (j + 1)]
                st_, sp_ = (i == 0), (i == TCH - 1)
                nc.tensor.matmul(po0[:], lhsT, xt[:, i, 0:512], start=st_, stop=sp_)
                nc.tensor.matmul(po1[:], lhsT, xt[:, i, 512:1024], start=st_, stop=sp_)
                nc.tensor.matmul(zp[:], lhsT, ones_r[:], start=st_, stop=sp_)
            rz = scrp.tile([128, 1], F32, tag="rz")
            nc.vector.reciprocal(rz[:], zp[:, 0:1])
            ob = outp.tile([128, E], F32)
            nc.scalar.mul(ob[:, 0:512], po0[:], rz[:])
            nc.scalar.mul(ob[:, 512:1024], po1[:], rz[:])
            nc.sync.dma_start(out_d[128 * j : 128 * (j + 1), :], ob[:])


def build_kernel():
    nc = bacc.Bacc(
        "TRN2",
        target_bir_lowering=False,
        debug=False,
        num_devices=N_CORES,
    )
    # x is declared MM_DT (same 4-byte layout as f32) so the HWDGE load
    # into the f32r-typed xt tile is cast-free and verifier-consistent.
    x_d = nc.dram_tensor("x", [T, E], MM_DT, kind="ExternalInput").ap()
    w_d = nc.dram_tensor("w", [E], F32, kind="ExternalInput").ap()
    b_d = nc.dram_tensor("b", [1], F32, kind="ExternalInput").ap()
    st_d = nc.dram_tensor("start", [SQ], I32, kind="ExternalInput").ap()
    en_d = nc.dram_tensor("end", [SQ], I32, kind="ExternalInput").ap()
    out_d = nc.dram_tensor("out", [SQ, E], F32, kind="ExternalOutput").ap()

    with tile.TileContext(nc) as tc:
        _build_body(tc, out_d, x_d, w_d, b_d, st_d, en_d)
    nc.compile()
    return nc


_NC_CACHE = None


def _get_nc():
    global _NC_CACHE
    if _NC_CACHE is None:
        _NC_CACHE = build_kernel()
    return _NC_CACHE


def _make_in_maps(x, W, b, start, end):
    x = np.asarray(x, dtype=np.float32)
    w_flat = np.ascontiguousarray(np.asarray(W, dtype=np.float32).reshape(E))
    b_arr = np.ascontiguousarray(np.asarray(b, dtype=np.float32).reshape(1))
    start = np.asarray(start, dtype=np.int32)
    end = np.asarray(end, dtype=np.int32)
    in_maps = []
    for core in range(N_CORES):
        bb, qq = divmod(core, 4)
        in_maps.append(
            {
                "x": np.ascontiguousarray(x[bb]),
                "w": w_flat,
                "b": b_arr,
                "start": np.ascontiguousarray(start[qq * SQ : (qq + 1) * SQ]),
                "end": np.ascontiguousarray(end[qq * SQ : (qq + 1) * SQ]),
            }
        )
    return in_maps


def run(x, W, b, start, end, trace=False, trace_cores=None):
    """Run on 8 cores; returns (out[B,S,E] f32, BassKernelResults)."""
    nc = _get_nc()
    in_maps = _make_in_maps(x, W, b, start, end)
    res = bass_utils.run_bass_kernel_spmd(
        nc,
        in_maps,
        core_ids=list(range(N_CORES)),
        trace=trace,
        trace_cores=trace_cores,
    )
    out = np.empty((B, S, E), np.float32)
    for core in range(N_CORES):
        bb, qq = divmod(core, 4)
        out[bb, qq * SQ : (qq + 1) * SQ] = res.results[core]["out"]
    return out, res


def kernel(x, W, b, start, end):
    out, _ = run(x, W, b, start, end, trace=False)
    return out


# revision 13
# speedup vs baseline: 1.2366x; 1.2366x over previous
"""Trainium2 Bass kernel for span-attention pooling.

Problem shapes (hardcoded):
  x: [B=2, T=512, E=1024] f32, W: [1024, 1] f32, b: [1] f32,
  start/end: [S=2048] i32.  Output: [B, S, E] f32.

Math: out[b,s,:] = sum_{t=start[s]}^{end[s]} q[b,t] * x[b,t,:] / sum q[b,t]
with q = exp(relu(x @ W + b)).  (Equivalent to the reference's per-span
softmax over head scores, since spans are contiguous token ranges and
clamped/invalid positions carry zero weight.)

Sharding: 8 cores = (batch b in {0,1}) x (span quarter of 512 spans).
Each core computes out[b, q*512:(q+1)*512, :].  The small tensors
(W, b, start, end) are host-replicated across the 128 partitions so the
kernel needs no on-chip broadcasts (gpsimd custom ops pay a ~10us
library-load penalty on this runtime).
"""

import numpy as np

import concourse.bass as bass
import concourse.tile as tile
from concourse import bacc, mybir
from concourse import bass_utils

B, T, E = 2, 512, 1024
S, A = 2048, 30
N_CORES = 8
SQ = S // 4  # spans per core
TCH = T // 128  # T chunks of 128 partitions
SCH = SQ // 128  # span chunks of 128 partitions

F32 = mybir.dt.float32
F32R = mybir.dt.float32r
I32 = mybir.dt.int32

# dtype for the main matmul operands. float32r runs the PE at ~2x the
# fp32 rate with ~tf32 precision (1.6e-4 rel, HW-measured). The walrus
# verifier requires f32r matmul inputs to be *produced* as f32r, so the
# tiles feeding the matmul are typed MM_DT and bitcast to f32 for DVE.
MM_DT = F32R


def _f32view(ap):
    if MM_DT == F32:
        return ap
    return ap.bitcast(F32)


def _build_body(tc, out_d, x_d, wb_d, bb_d, stb_d, enb_d, tcol_d):
    nc = tc.nc
    AF = mybir.ActivationFunctionType
    OP = mybir.AluOpType

    with (
        tc.tile_pool(name="main", bufs=1) as mainp,
        tc.tile_pool(name="outp", bufs=2) as outp,
        tc.tile_pool(name="psum", bufs=1, space="PSUM") as psp,
        tc.tile_pool(name="scr", bufs=1) as scrp,
    ):
        # x chunks on the Sync DMA ring (chunk 0 first: the DVE head
        # pipeline starts as soon as it lands).
        xts = []
        for i in range(TCH):
            xt = mainp.tile([128, E], MM_DT, tag=f"xt{i}")
            nc.sync.dma_start(xt[:], x_d[128 * i : 128 * (i + 1), :])
            xts.append(xt)

        # Small replicated loads on the Scalar HWDGE ring (parallel with x).
        wb = mainp.tile([128, E], F32)
        nc.scalar.dma_start(wb[:], wb_d[:])
        bb = mainp.tile([128, 1], F32)
        nc.scalar.dma_start(bb[:], bb_d[:])
        tcol = mainp.tile([128, TCH], F32)
        nc.scalar.dma_start(tcol[:], tcol_d[:])
        stb_i = mainp.tile([128, SQ], I32)
        nc.scalar.dma_start(stb_i[:], stb_d[:])
        enb_i = mainp.tile([128, SQ], I32)
        nc.scalar.dma_start(enb_i[:], enb_d[:])

        # int -> float casts (values < 2^24, exact).
        stb = mainp.tile([128, SQ], F32)
        nc.vector.tensor_copy(stb[:], stb_i[:])
        enb = mainp.tile([128, SQ], F32)
        nc.vector.tensor_copy(enb[:], enb_i[:])

        # ones tile for the Z (softmax denominator) matmul. 64 wide:
        # f32r matmuls reject tiny moving free dims (N=1 fails the ISA
        # check). memset can't write f32r -> memset f32 then copy.
        ones_f = mainp.tile([128, 64], F32)
        nc.vector.memset(ones_f[:], 1.0)
        ones_r = mainp.tile([128, 64], MM_DT)
        nc.vector.tensor_copy(ones_r[:], ones_f[:])

        # Per T-chunk: head score h, q = exp(relu(h + b)), and
        # maskq[t, s] = (start[s] <= t <= end[s]) * q[t]  (transposed layout).
        q_col = mainp.tile([128, TCH], F32)
        rh = mainp.tile([128, TCH], F32)
        h = mainp.tile([128, TCH], F32)
        scr = scrp.tile([128, E], F32)
        m2 = scrp.tile([128, SQ], F32)
        mqs = []
        for i in range(TCH):
            # h = sum_e x[t, e] * W[e]
            nc.vector.scalar_tensor_tensor(
                scr[:],
                _f32view(xts[i][:]),
                1.0,
                wb[:],
                op0=OP.mult,
                op1=OP.mult,
                accum_out=h[:, i : i + 1],
            )
            # q = exp(relu(h + b))  (two ScalarE ops, bias folded in)
            nc.scalar.activation(
                rh[:, i : i + 1], h[:, i : i + 1], AF.Relu, bias=bb[:, 0:1]
            )
            nc.scalar.activation(q_col[:, i : i + 1], rh[:, i : i + 1], AF.Exp)
            # m2 = (end >= t) * q
            nc.vector.tensor_scalar(
                m2[:],
                enb[:],
                tcol[:, i : i + 1],
                q_col[:, i : i + 1],
                op0=OP.is_ge,
                op1=OP.mult,
            )
            # maskq = (start <= t) * m2
            mq = mainp.tile([128, SQ], MM_DT, tag=f"mq{i}")
            nc.vector.scalar_tensor_tensor(
                mq[:],
                stb[:],
                tcol[:, i : i + 1],
                m2[:],
                op0=OP.is_le,
                op1=OP.mult,
            )
            mqs.append(mq)

        # Matmuls, i-major in two passes of two span-chunks (PSUM: 3
        # banks per span-chunk x 2 in flight = 6 of 8 banks):
        #   out_psum[s, e] = sum_t maskq[t, s] * x[t, e]
        #   Z[s]           = sum_t maskq[t, s]
        for jj in (0, 2):
            pos = {}
            for j in (jj, jj + 1):
                pos[j] = (
                    psp.tile([128, 512], F32, name=f"po0_{j}", tag=f"po0_{j % 2}"),
                    psp.tile([128, 512], F32, name=f"po1_{j}", tag=f"po1_{j % 2}"),
                    psp.tile([128, 64], F32, name=f"zp_{j}", tag=f"zp_{j % 2}"),
                )
            for i in range(TCH):
                st_, sp_ = (i == 0), (i == TCH - 1)
                for j in (jj, jj + 1):
                    lhsT = mqs[i][:, 128 * j : 128 * (j + 1)]
                    po0, po1, zp = pos[j]
                    nc.tensor.matmul(po0[:], lhsT, xts[i][:, 0:512], start=st_, stop=sp_)
                    nc.tensor.matmul(po1[:], lhsT, xts[i][:, 512:1024], start=st_, stop=sp_)
                    nc.tensor.matmul(zp[:], lhsT, ones_r[:], start=st_, stop=sp_)
            for j in (jj, jj + 1):
                po0, po1, zp = pos[j]
                rz = scrp.tile([128, 1], F32, tag=f"rz{j % 2}")
                nc.vector.reciprocal(rz[:], zp[:, 0:1])
                ob = outp.tile([128, E], F32)
                # normalization split across ScalarE and VectorE
                nc.scalar.mul(ob[:, 0:512], po0[:], rz[:])
                nc.vector.tensor_scalar_mul(ob[:, 512:1024], po1[:], rz[:])
                nc.sync.dma_start(out_d[128 * j : 128 * (j + 1), :], ob[:])


def build_kernel():
    nc = bacc.Bacc(
        "TRN2",
        target_bir_lowering=False,
        debug=False,
        num_devices=N_CORES,
    )
    # x is declared MM_DT (same 4-byte layout as f32) so the HWDGE load
    # into the f32r-typed xt tiles is cast-free and verifier-consistent.
    x_d = nc.dram_tensor("x", [T, E], MM_DT, kind="ExternalInput").ap()
    wb_d = nc.dram_tensor("wb", [128, E], F32, kind="ExternalInput").ap()
    bb_d = nc.dram_tensor("bb", [128, 1], F32, kind="ExternalInput").ap()
    stb_d = nc.dram_tensor("stb", [128, SQ], I32, kind="ExternalInput").ap()
    enb_d = nc.dram_tensor("enb", [128, SQ], I32, kind="ExternalInput").ap()
    tcol_d = nc.dram_tensor("tcol", [128, TCH], F32, kind="ExternalInput").ap()
    out_d = nc.dram_tensor("out", [SQ, E], F32, kind="ExternalOutput").ap()

    with tile.TileContext(nc) as tc:
        _build_body(tc, out_d, x_d, wb_d, bb_d, stb_d, enb_d, tcol_d)
    nc.compile()
    return nc


_NC_CACHE = None


def _get_nc():
    global _NC_CACHE
    if _NC_CACHE is None:
        _NC_CACHE = build_kernel()
    return _NC_CACHE


def _make_in_maps(x, W, b, start, end):
    x = np.asarray(x, dtype=np.float32)
    start = np.asarray(start, dtype=np.int32)
    end = np.asarray(end, dtype=np.int32)
    wb = np.ascontiguousarray(
        np.broadcast_to(np.asarray(W, np.float32).reshape(1, E), (128, E))
    )
    bb = np.ascontiguousarray(
        np.broadcast_to(np.asarray(b, np.float32).reshape(1, 1), (128, 1))
    )
    tcol = (
        np.arange(128, dtype=np.float32)[:, None]
        + 128.0 * np.arange(TCH, dtype=np.float32)[None, :]
    ).astype(np.float32)
    tcol = np.ascontiguousarray(tcol)
    in_maps = []
    for core in range(N_CORES):
        bb_idx, qq = divmod(core, 4)
        st_q = start[qq * SQ : (qq + 1) * SQ]
        en_q = end[qq * SQ : (qq + 1) * SQ]
        in_maps.append(
            {
                "x": np.ascontiguousarray(x[bb_idx]),
                "wb": wb,
                "bb": bb,
                "stb": np.ascontiguousarray(np.broadcast_to(st_q[None, :], (128, SQ))),
                "enb": np.ascontiguousarray(np.broadcast_to(en_q[None, :], (128, SQ))),
                "tcol": tcol,
            }
        )
    return in_maps


def run(x, W, b, start, end, trace=False, trace_cores=None):
    """Run on 8 cores; returns (out[B,S,E] f32, BassKernelResults)."""
    nc = _get_nc()
    in_maps = _make_in_maps(x, W, b, start, end)
    res = bass_utils.run_bass_kernel_spmd(
        nc,
        in_maps,
        core_ids=list(range(N_CORES)),
        trace=trace,
        trace_cores=trace_cores,
    )
    out = np.empty((B, S, E), np.float32)
    for core in range(N_CORES):
        bb_idx, qq = divmod(core, 4)
        out[bb_idx, qq * SQ : (qq + 1) * SQ] = res.results[core]["out"]
    return out, res


def kernel(x, W, b, start, end):
    out, _ = run(x, W, b, start, end, trace=False)
    return out
